# revision 1
# baseline (speedup 1.0000x reference)
"""Trainium2 Bass kernel for the DMN EpisodicMemoryModule.

Strategy (8 NeuronCores, data-parallel over batch):
  - Each core processes B_loc = 16 of the 128 samples; weights replicated.
  - All on-chip tensors live "transposed" ([U, batch]) so the sequential
    attention-GRU scan needs no per-step transposes: matmuls contract over
    U on partitions, elementwise ops run on [128, 2, B_loc] tiles.
  - Matmul operands are fp16 (PSUM accumulates fp32); softmax, gates and
    the memory state stay fp32.
  - Per memory step: score GEMM (4 fused z-components x l1_W) -> tanh ->
    l2 projection -> softmax -> gate broadcast -> 512-step scan -> memory
    update GEMM.
"""

import os
import sys
import numpy as np

try:
    import concourse.bass as _probe  # noqa: F401
except ImportError:  # fresh grading dir: concourse repo may not be on sys.path
    for _p in ("/opt/trn_rl_repo", "/opt/pypackages",
               "/root/.axon_site/_ro/trn_rl_repo", "/root/.axon_site/_ro/pypackages"):
        if os.path.isdir(_p) and _p not in sys.path:
            sys.path.append(_p)

import concourse.bass as bass
import concourse.mybir as mybir
from concourse import bacc
import concourse.tile as tile
from concourse.bass import ts
from concourse.masks import make_identity

P = 128
B, T, U, EMB = 128, 512, 256, 256
MEM_STEPS = 3
NCORES = 8
BL = B // NCORES  # 16 samples per core
UC = U // P       # 2 partition chunks of U
EC = EMB // P     # 2 partition chunks of EMB
TO = T // P       # 4 t-chunks of 128

f32 = mybir.dt.float32
f16 = mybir.dt.float16
AF = mybir.ActivationFunctionType
ALU = mybir.AluOpType
AX = mybir.AxisListType


def build_kernel(bl=BL, t_len=T, mem_steps=MEM_STEPS):
    """Build the single-core Bass module. bl/t_len/mem_steps shrinkable for sim."""
    to = t_len // P
    nc = bacc.Bacc(trn_type="TRN2")

    facts_d = nc.dram_tensor("facts", [bl, t_len, U], f32, kind="ExternalInput")
    question_d = nc.dram_tensor("question", [bl, U], f32, kind="ExternalInput")
    l1W_d = nc.dram_tensor("l1_W", [4 * U, EMB], f32, kind="ExternalInput")
    l1b_d = nc.dram_tensor("l1_b", [EMB], f32, kind="ExternalInput")
    l2W_d = nc.dram_tensor("l2_W", [EMB, 1], f32, kind="ExternalInput")
    Wr_d = nc.dram_tensor("Wr", [U, U], f32, kind="ExternalInput")
    Ur_d = nc.dram_tensor("Ur", [U, U], f32, kind="ExternalInput")
    br_d = nc.dram_tensor("br", [U], f32, kind="ExternalInput")
    Wh_d = nc.dram_tensor("Wh", [U, U], f32, kind="ExternalInput")
    Uh_d = nc.dram_tensor("Uh", [U, U], f32, kind="ExternalInput")
    bh_d = nc.dram_tensor("bh", [U], f32, kind="ExternalInput")
    memW_d = nc.dram_tensor("mem_W", [3 * U, U], f32, kind="ExternalInput")
    memb_d = nc.dram_tensor("mem_b", [U], f32, kind="ExternalInput")
    out_d = nc.dram_tensor("out", [bl, 2 * U], f32, kind="ExternalOutput")

    with tile.TileContext(nc) as tc:
        with (
            tc.tile_pool(name="persist", bufs=1) as pp,
            tc.tile_pool(name="wload", bufs=1) as lp,
            tc.tile_pool(name="work", bufs=3) as wp,
            tc.tile_pool(name="comp", bufs=2) as cp,
            tc.tile_pool(name="psum_big", bufs=2, space="PSUM") as pb,
            tc.tile_pool(name="psum_small", bufs=3, space="PSUM") as psc,
        ):
            # ---------------- weights / constants into SBUF ----------------
            id32 = pp.tile([P, P], f32)
            make_identity(nc, id32[:])
            id16 = pp.tile([P, P], f16)
            nc.vector.tensor_copy(id16[:], id32[:])
            ones16 = pp.tile([1, P], f16)
            nc.vector.memset(ones16[:], 1.0)

            def load_w16(dram, rows, name):
                ko = rows // P
                w16 = pp.tile([P, ko, dram.shape[1]], f16, name=name, tag=name)
                nc.gpsimd.dma_start(w16[:], dram.rearrange("(ko p) m -> p ko m", p=P))
                return w16

            wr16 = load_w16(Wr_d, U, "wr16")
            ur16 = load_w16(Ur_d, U, "ur16")
            wh16 = load_w16(Wh_d, U, "wh16")
            uh16 = load_w16(Uh_d, U, "uh16")
            l1w16 = load_w16(l1W_d, 4 * U, "l1w16")   # [128, 8, 256]
            memw16 = load_w16(memW_d, 3 * U, "memw16")  # [128, 6, 256]
            l2w16 = load_w16(l2W_d, EMB, "l2w16")     # [128, 2, 1]

            # biases as [128, chunks] columns (slow elementwise DMA, tiny)
            l1b_sb = pp.tile([P, EC], f32)
            nc.gpsimd.dma_start(l1b_sb[:], l1b_d.rearrange("(c p) -> p c", p=P))
            br_sb = pp.tile([P, UC], f32)
            nc.gpsimd.dma_start(br_sb[:], br_d.rearrange("(c p) -> p c", p=P))
            bh_sb = pp.tile([P, UC], f32)
            nc.gpsimd.dma_start(bh_sb[:], bh_d.rearrange("(c p) -> p c", p=P))
            memb_sb = pp.tile([P, UC], f32)
            nc.gpsimd.dma_start(memb_sb[:], memb_d.rearrange("(c p) -> p c", p=P))

            # question transposed [128, UC, bl] (elementwise DMA, 16KB once)
            qT = pp.tile([P, UC, bl], f32)
            for uc in range(UC):
                nc.gpsimd.dma_start(
                    qT[:, uc, :],
                    question_d[:, ts(uc, P)].rearrange("b p -> p b"),
                )
            qT16 = pp.tile([P, UC, bl], f16)
            nc.scalar.copy(qT16[:], qT[:])
            qTneg = pp.tile([P, UC, bl], f32)
            nc.vector.tensor_scalar_mul(qTneg[:], qT[:], -1.0)

            # ---------------- facts load + transpose -> factsT fp16 ----------------
            factsT = pp.tile([P, UC, bl, t_len], f16)  # 32KB/partition
            for b in range(bl):
                bounce = wp.tile([P, to, U], f32, tag="fbounce")
                nc.gpsimd.dma_start(
                    bounce[:], facts_d[b].rearrange("(to p) u -> p to u", p=P)
                )
                for toi in range(to):
                    for uc in range(UC):
                        pt = pb.tile([P, P], f32, tag="big")
                        nc.tensor.transpose(pt[:], bounce[:, toi, ts(uc, P)], id32[:])
                        eng = nc.scalar if (toi + uc) % 2 == 0 else nc.vector
                        if eng is nc.scalar:
                            nc.scalar.copy(factsT[:, uc, b, ts(toi, P)], pt[:])
                        else:
                            nc.vector.tensor_copy(factsT[:, uc, b, ts(toi, P)], pt[:])

            # ---------------- XR / XH = (facts @ W + b)^T, fp16 ----------------
            # layout [128, UC(mo), t, bl]
            XR = pp.tile([P, UC, t_len, bl], f16)
            XH = pp.tile([P, UC, t_len, bl], f16)
            for dst, w16, bias in ((XR, wr16, br_sb), (XH, wh16, bh_sb)):
                for mo in range(UC):
                    for b in range(bl):
                        px = pb.tile([P, t_len], f32, tag="big")
                        for ko in range(UC):
                            nc.tensor.matmul(
                                px[:], w16[:, ko, ts(mo, P)], factsT[:, ko, b, :],
                                start=(ko == 0), stop=(ko == UC - 1),
                            )
                        o = dst[:, mo, :, b]
                        if b % 2 == 0:
                            nc.scalar.add(o, px[:], bias[:, mo : mo + 1])
                        else:
                            nc.vector.tensor_scalar_add(o, px[:], bias[:, mo : mo + 1])

            # ---------------- persistent states ----------------
            mT = pp.tile([P, UC, bl], f32)    # memory^T
            nc.vector.tensor_copy(mT[:], qT[:])
            mT16 = pp.tile([P, UC, bl], f16)
            nc.vector.tensor_copy(mT16[:], qT16[:])
            mTneg = pp.tile([P, UC, bl], f32)
            NG = 1  # independent scan groups per core (2 measured slower: overhead-bound)
            Hg = [pp.tile([P, UC, bl // NG], f16, name=f"H16_{g}", tag=f"H16_{g}")
                  for g in range(NG)]  # GRU hidden state per group
            G = pp.tile([P, t_len, bl], f16)  # attention gates, replicated over partitions
            scores_sb = pp.tile([32, t_len], f32)  # only first bl partitions used

            # ---------------- memory iterations ----------------
            for step in range(mem_steps):
                nc.vector.tensor_scalar_mul(mTneg[:], mT[:], -1.0)

                # --- scores GEMM over 4 z-components, streamed per sample ---
                tanh_tiles = []
                for b in range(bl):
                    # component tiles [128, UC, t_len] fp16
                    cq = cp.tile([P, UC, t_len], f16, tag="cq")
                    cm = cp.tile([P, UC, t_len], f16, tag="cm")
                    aq = cp.tile([P, UC, t_len], f16, tag="aq")
                    am = cp.tile([P, UC, t_len], f16, tag="am")
                    d16 = cp.tile([P, UC, t_len], f16, tag="d16")
                    for uc in range(UC):
                        fT = factsT[:, uc, b, :]
                        nc.gpsimd.tensor_scalar_mul(cq[:, uc, :], fT, qT[:, uc, b : b + 1])
                        nc.gpsimd.tensor_scalar_mul(cm[:, uc, :], fT, mT[:, uc, b : b + 1])
                        nc.scalar.activation(
                            aq[:, uc, :], fT, AF.Abs, bias=qTneg[:, uc, b : b + 1]
                        )
                        nc.vector.tensor_scalar_add(
                            d16[:, uc, :], fT, mTneg[:, uc, b : b + 1]
                        )
                        nc.vector.scalar_tensor_tensor(
                            am[:, uc, :], d16[:, uc, :], -1.0, d16[:, uc, :],
                            ALU.mult, ALU.max,
                        )
                    comps = (cq, cm, aq, am)
                    tanhE = cp.tile([P, EC, t_len], f16, tag="tanhE")
                    for eo in range(EC):
                        ps = pb.tile([P, t_len], f32, tag="big")
                        n_mm = 4 * UC
                        i = 0
                        for c in range(4):
                            for ko in range(UC):
                                nc.tensor.matmul(
                                    ps[:],
                                    l1w16[:, 2 * c + ko, ts(eo, P)],
                                    comps[c][:, ko, :],
                                    start=(i == 0), stop=(i == n_mm - 1),
                                )
                                i += 1
                        nc.scalar.activation(
                            tanhE[:, eo, :], ps[:], AF.Tanh, bias=l1b_sb[:, eo : eo + 1]
                        )
                    # l2 projection -> scores[b, :] via PSUM partition 0
                    pl2 = psc.tile([1, t_len], f32, tag="s2")
                    for eo in range(EC):
                        nc.tensor.matmul(
                            pl2[:], l2w16[:, eo, :], tanhE[:, eo, :],
                            start=(eo == 0), stop=(eo == EC - 1),
                        )
                    sc_b = wp.tile([1, t_len], f32, tag="sc_b")
                    nc.scalar.copy(sc_b[:], pl2[:])
                    nc.gpsimd.dma_start(scores_sb[b : b + 1, :], sc_b[:])

                # --- softmax over t (rows 0..bl-1) ---
                mx = wp.tile([32, 1], f32, tag="mx")
                nc.vector.tensor_reduce(
                    mx[:bl], scores_sb[:bl], axis=AX.X, op=ALU.max
                )
                negmx = wp.tile([32, 1], f32, tag="negmx")
                nc.vector.tensor_scalar_mul(negmx[:bl], mx[:bl], -1.0)
                exps = wp.tile([32, t_len], f32, tag="exps")
                sume = wp.tile([32, 1], f32, tag="sume")
                nc.scalar.activation(
                    exps[:bl], scores_sb[:bl], AF.Exp,
                    bias=negmx[:bl], accum_out=sume[:bl],
                )
                rinv = wp.tile([32, 1], f32, tag="rinv")
                nc.vector.reciprocal(rinv[:bl], sume[:bl])
                att16 = wp.tile([32, t_len], f16, tag="att16")
                nc.vector.tensor_scalar_mul(att16[:bl], exps[:bl], rinv[:bl])

                # --- broadcast gates to all partitions: G[p, t, b] = att[b, t] ---
                tch = 512 // bl  # t-chunk so N = tch*bl = 512
                for tc_i in range(t_len // tch):
                    g_src = wp.tile([1, bl, tch], f16, tag="g_src")
                    nc.gpsimd.dma_start(
                        g_src[:], att16[:bl, tc_i * tch : (tc_i + 1) * tch]
                    )
                    pg = pb.tile([P, tch, bl], f32, tag="big")
                    nc.tensor.matmul(
                        pg[:], ones16[:], g_src.rearrange("o b t -> o t b"),
                        start=True, stop=True,
                    )
                    o = G[:, tc_i * tch : (tc_i + 1) * tch, :]
                    if tc_i % 2 == 0:
                        nc.scalar.copy(o, pg[:])
                    else:
                        nc.vector.tensor_copy(o, pg[:])

                # --- the sequential attention-GRU scan ---
                # Two independent sample groups per core: the serial
                # per-step chain (PE->ACT->DVE->PE->ACT->DVE) of one group
                # overlaps the other group's, hiding semaphore-hop latency.
                gbl = bl // NG
                for g in range(NG):
                    nc.vector.memset(Hg[g][:], 0.0)
                for t in range(t_len):
                    s1g = [psc.tile([P, UC, gbl], f32, tag="s1", name=f"s1_{g}")
                           for g in range(NG)]
                    for mo in range(UC):
                        for g in range(NG):
                            nc.tensor.matmul(
                                s1g[g][:, mo, :], id16[:],
                                XR[:, mo, t, ts(g, gbl)],
                                start=True, stop=False, skip_group_check=True,
                            )
                        for ko in range(UC):
                            for g in range(NG):
                                nc.tensor.matmul(
                                    s1g[g][:, mo, :], ur16[:, ko, ts(mo, P)],
                                    Hg[g][:, ko, :],
                                    start=False, stop=(ko == UC - 1),
                                    skip_group_check=True,
                                )
                    qg = []
                    for g in range(NG):
                        r16 = wp.tile([P, UC, gbl], f16, tag=f"r16_{g}")
                        nc.scalar.activation(r16[:], s1g[g][:], AF.Sigmoid)
                        q16 = wp.tile([P, UC, gbl], f16, tag=f"q16_{g}")
                        nc.vector.tensor_mul(q16[:], r16[:], Hg[g][:])
                        qg.append(q16)

                    s2g = [psc.tile([P, UC, gbl], f32, tag="s2", name=f"s2_{g}")
                           for g in range(NG)]
                    for mo in range(UC):
                        for g in range(NG):
                            nc.tensor.matmul(
                                s2g[g][:, mo, :], id16[:],
                                XH[:, mo, t, ts(g, gbl)],
                                start=True, stop=False, skip_group_check=True,
                            )
                        for ko in range(UC):
                            for g in range(NG):
                                nc.tensor.matmul(
                                    s2g[g][:, mo, :], uh16[:, ko, ts(mo, P)],
                                    qg[g][:, ko, :],
                                    start=False, stop=(ko == UC - 1),
                                    skip_group_check=True,
                                )
                    for g in range(NG):
                        ht16 = wp.tile([P, UC, gbl], f16, tag=f"ht16_{g}")
                        nc.scalar.activation(ht16[:], s2g[g][:], AF.Tanh)
                        # H += g * (ht - H)
                        dd = wp.tile([P, UC, gbl], f16, tag=f"dd_{g}")
                        nc.vector.tensor_sub(dd[:], ht16[:], Hg[g][:])
                        ee = wp.tile([P, UC, gbl], f16, tag=f"ee_{g}")
                        gt = G[:, t : t + 1, ts(g, gbl)].to_broadcast([P, UC, gbl])
                        nc.vector.tensor_mul(ee[:], dd[:], gt)
                        nc.vector.tensor_add(Hg[g][:], ee[:], Hg[g][:])

                # --- memory update: mT = relu(memW^T @ [m; episode; q] + memb) ---
                pm = psc.tile([P, UC, bl], f32, tag="s1")
                for gi in range(NG):
                    gs = ts(gi, bl // NG)
                    rhs_k = [mT16[:, 0, gs], mT16[:, 1, gs],
                             Hg[gi][:, 0, :], Hg[gi][:, 1, :],
                             qT16[:, 0, gs], qT16[:, 1, gs]]
                    for mo in range(UC):
                        for ko in range(6):
                            nc.tensor.matmul(
                                pm[:, mo, gs], memw16[:, ko, ts(mo, P)], rhs_k[ko],
                                start=(ko == 0), stop=(ko == 5),
                                skip_group_check=True,
                            )
                for mo in range(UC):
                    nc.scalar.activation(
                        mT[:, mo, :], pm[:, mo, :], AF.Relu,
                        bias=memb_sb[:, mo : mo + 1],
                    )
                nc.scalar.copy(mT16[:], mT[:])

            # ---------------- output: [memory, question] ----------------
            out_nat = wp.tile([32, UC, P], f32, tag="outnat")
            for mo in range(UC):
                po = pb.tile([P, P], f32, tag="big")
                nc.tensor.transpose(po[:bl, :], mT[:, mo, :], id32[:])
                nc.scalar.copy(out_nat[:bl, mo, :], po[:bl, :])
            nc.gpsimd.dma_start(out_d[:, 0:U], out_nat[:bl])
            nc.gpsimd.dma_start(out_d[:, U : 2 * U], question_d[:])

    nc.finalize()
    return nc


_NC_CACHE = {}


def _get_nc():
    key = (BL, T, MEM_STEPS)
    if key not in _NC_CACHE:
        _NC_CACHE[key] = build_kernel()
    return _NC_CACHE[key]


def kernel(**inputs):
    from concourse.bass_utils import run_bass_kernel_spmd

    nc = _get_nc()
    names = ["facts", "question", "l1_W", "l1_b", "l2_W", "Wr", "Ur", "br",
             "Wh", "Uh", "bh", "mem_W", "mem_b"]
    full = {k: np.ascontiguousarray(np.asarray(inputs[k]), dtype=np.float32)
            for k in names}
    in_maps = []
    for c in range(NCORES):
        m = dict(full)
        m["facts"] = np.ascontiguousarray(full["facts"][c * BL : (c + 1) * BL])
        m["question"] = np.ascontiguousarray(full["question"][c * BL : (c + 1) * BL])
        in_maps.append(m)
    res = run_bass_kernel_spmd(nc, in_maps, core_ids=list(range(NCORES)))
    return np.concatenate([r["out"] for r in res.results], axis=0)

